# revision 54
# baseline (speedup 1.0000x reference)
"""Cross-attention (single-head, residual) Bass/Tile kernel for Trainium2.

Problem: y = x + (softmax((x' Wq + bq)(ctx Wk + bk)^T / sqrt(C)) (ctx Wv + bv)) Wo + bo
  x: [B=8, C=512, H=64, W=64], context: [B=8, Lc=512, CTX=768]

Sharding: pure data-parallel over batch — one batch element per NeuronCore,
no collectives.

Algebraic restructuring (saves ~1/3 of the matmul work): with
  kT = (ctx Wk)^T            [C, Lc]
  G  = Wq kT                 [C, Lc]   (Wq folded into the key side)
  vW = (ctx Wv + bv) Wo      [Lc, C]   (Wo folded into the value side)
the streaming loop per hw-tile is only two matmul stages:
  simT = G^T-contracted-with-x:  simT[lc,hw] = sum_c' x[c',hw] G[c',lc]
  eT   = exp(scale*simT + scale*(kT^T bq))        (bq folded into ACT bias)
  y    = (vW^T eT) * (1/colsum(eT)) + bo + x      (softmax denom folded in)
bv is exact under the fold because softmax rows sum to 1.

Dtype strategy: HBM-resident tensors stay fp32 and feed the PE as float32r
(full rate at free-dim 512) — no cast passes exist.  Engine-produced matmul
operands are written f32r or bf16 during their mandatory PSUM evictions.
Accumulation is always fp32 in PSUM; residual add and output are fp32.
"""

import numpy as np

B = 8
C = 512
CTX = 768
Lc = 512
HH = 64
WW = 64
HW = HH * WW          # 4096
N_CORES = 8
P = 128
HT = 512              # hw tile (free-dim) width
N_HT = HW // HT       # 8
KC = C // P           # 4
KX = CTX // P         # 6
KL = Lc // P          # 4
SCALE = float(C) ** -0.5

_cache = {}


def _build_nc():
    import concourse.mybir as mybir
    import concourse.bass as bass
    import concourse.tile as tile
    from concourse import bacc
    from concourse.masks import make_identity

    f32 = mybir.dt.float32
    f32r = mybir.dt.float32r
    bf16 = mybir.dt.bfloat16
    fp8 = mybir.dt.float8e4
    AF = mybir.ActivationFunctionType
    DR = mybir.MatmulPerfMode.DoubleRow

    nc = bacc.Bacc("TRN2", target_bir_lowering=False, debug=False,
                   num_devices=N_CORES)

    x_d = nc.dram_tensor("x", [C, HW], f32r, kind="ExternalInput").ap()
    ctx_d = nc.dram_tensor("ctx", [Lc, CTX], f32, kind="ExternalInput").ap()
    wq_d = nc.dram_tensor("wq", [C, C], f32r, kind="ExternalInput").ap()
    wk_d = nc.dram_tensor("wk", [CTX, C], f32r, kind="ExternalInput").ap()
    wv_d = nc.dram_tensor("wv", [CTX, C], f32r, kind="ExternalInput").ap()
    wo_d = nc.dram_tensor("wo", [C, C], f32r, kind="ExternalInput").ap()
    bq_d = nc.dram_tensor("bq", [C], f32, kind="ExternalInput").ap()
    bk_d = nc.dram_tensor("bk", [C], f32, kind="ExternalInput").ap()
    bv_d = nc.dram_tensor("bv", [C], f32, kind="ExternalInput").ap()
    bo_d = nc.dram_tensor("bo", [C], f32, kind="ExternalInput").ap()
    y_d = nc.dram_tensor("y", [C, HW], f32, kind="ExternalOutput").ap()

    x_r = x_d.rearrange("(ko p) hw -> p ko hw", p=P)      # [128, 4, 4096]
    y_r = y_d.rearrange("(ko p) hw -> p ko hw", p=P)
    ctx_r = ctx_d.rearrange("(lo p) cx -> p lo cx", p=P)  # [128, 4, 768]
    wq_r = wq_d.rearrange("(ko p) c -> p ko c", p=P)      # [128, 4, 512]
    wk_r = wk_d.rearrange("(ko p) c -> p ko c", p=P)      # [128, 6, 512]
    wv_r = wv_d.rearrange("(ko p) c -> p ko c", p=P)
    wo_r = wo_d.rearrange("(ko p) c -> p ko c", p=P)

    def r(ap):  # feed fp32 SBUF data to the PE at full rate
        return ap.bitcast(mybir.dt.float32r)

    with tile.TileContext(nc) as tc:
        with (
            tc.tile_pool(name="const", bufs=1) as const,
            tc.tile_pool(name="xin", bufs=4) as xin,
            tc.tile_pool(name="work", bufs=2) as work,
            tc.tile_pool(name="yout", bufs=2) as yout,
            tc.tile_pool(name="small", bufs=3) as small,
            tc.tile_pool(name="psum", bufs=3, space="PSUM") as psum,
            tc.tile_pool(name="psum_s", bufs=1, space="PSUM") as psum_s,
            tc.tile_pool(name="psum_bc", bufs=1, space="PSUM") as psum_bc,
        ):
            # ---------------- DMAs (ordered: ctx feeds the PE first) --------
            ctx_f = const.tile([P, KL, CTX], f32, name="ctx_f", tag="ctx_f")
            for lo in range(KL):  # chunked so transposes start early
                nc.sync.dma_start(out=ctx_f[:, lo, :CTX // 2],
                                  in_=ctx_r[:, lo, :CTX // 2])
                nc.sync.dma_start(out=ctx_f[:, lo, CTX // 2:],
                                  in_=ctx_r[:, lo, CTX // 2:])
            wq_f = const.tile([P, KC, C], f32r, name="wq_f", tag="wq_f")
            wk_f = const.tile([P, KX, C], f32r, name="wk_f", tag="wk_f")
            wv_f = const.tile([P, KX, C], f32r, name="wv_f", tag="wv_f")
            wo_f = const.tile([P, KC, C], f32r, name="wo_f", tag="wo_f")
            nc.sync.dma_start(out=wq_f, in_=wq_r)
            nc.sync.dma_start(out=wk_f, in_=wk_r)
            nc.sync.dma_start(out=wv_f, in_=wv_r)
            nc.sync.dma_start(out=wo_f, in_=wo_r)

            x_tiles = {}
            for h in range(2):
                x_f = xin.tile([P, KC, HT], f32r, tag="x_f", name=f"x_f_{h}")
                nc.sync.dma_start(out=x_f, in_=x_r[:, :, h * HT:(h + 1) * HT])
                x_tiles[h] = x_f

            ident_f = const.tile([P, P], f32, name="ident_f", tag="ident")
            make_identity(nc, ident_f)
            ones_col = const.tile([P, 1], fp8, name="ones_col", tag="ones_c")
            nc.vector.memset(ones_col, 1.0)
            ones_row = const.tile([1, P], bf16, name="ones_row", tag="ones_r")
            nc.vector.memset(ones_row, 1.0)

            # biases (tiny scattered DMAs on the gpsimd queue; bq cast to fp8
            # by the gpsimd DGE so it pairs with the fp8 kT in the bqk matvec)
            bq_t = const.tile([P, KC], fp8, name="bq_t", tag="bq")
            bk_t = const.tile([P, KC], f32, name="bk_t", tag="bk")
            bv_t = const.tile([P, KC], f32, name="bv_t", tag="bv")
            with nc.allow_non_contiguous_dma(reason="tiny one-time bias loads"):
                nc.gpsimd.dma_start(out=bq_t, in_=bq_d.rearrange("(ko p) -> p ko", p=P))
                nc.gpsimd.dma_start(out=bk_t, in_=bk_d.rearrange("(ko p) -> p ko", p=P))
                nc.gpsimd.dma_start(out=bv_t, in_=bv_d.rearrange("(ko p) -> p ko", p=P))
            # bo broadcast across partitions (folded into vW: rows of attn sum
            # to 1, so attn @ (vW + 1 bo^T) = attn vW + bo exactly)
            bo_bc = const.tile([P, C], f32, name="bo_bc", tag="bo")
            bo_src = bass.AP(tensor=bo_d.tensor, offset=bo_d.offset,
                             ap=[[0, P]] + list(bo_d.ap))
            nc.gpsimd.dma_start(out=bo_bc, in_=bo_src)

            # ---------------- phase A ----------------
            # context transpose: ctxT [128(cx), KX, Lc]
            ctxT_f = const.tile([P, KX, Lc], f32, name="ctxT_f", tag="ctxT")
            for lo in range(KL):
                for cx in range(KX):
                    ps_t = psum.tile([P, P], f32, tag="mm", name=f"ps_t_{lo}_{cx}")
                    nc.tensor.transpose(ps_t, ctx_f[:, lo, cx * P:(cx + 1) * P],
                                        ident_f)
                    if cx % 2 == 0:
                        nc.scalar.activation(r(ctxT_f[:, cx, lo * P:(lo + 1) * P]),
                                             ps_t, AF.Copy)
                    else:
                        nc.vector.tensor_copy(
                            out=r(ctxT_f[:, cx, lo * P:(lo + 1) * P]), in_=ps_t)

            # Wq transpose: WqT [128(c), KC, C(c')], fp8
            wqT_8 = const.tile([P, KC, C], fp8, name="wqT_8", tag="wqT")
            for ko in range(KC):
                for mc in range(KC):
                    ps_t = psum.tile([P, P], f32, tag="mm", name=f"ps_w_{ko}_{mc}")
                    nc.tensor.transpose(
                        ps_t, wq_f[:, ko, mc * P:(mc + 1) * P].bitcast(f32), ident_f)
                    nc.vector.tensor_copy(
                        out=wqT_8[:, mc, ko * P:(ko + 1) * P], in_=ps_t)

            # kT [128(c), KC, Lc] = (ctx Wk + bk)^T, fp8 (K-side only shifts
            # logits; the V path stays full precision)
            kT_8 = const.tile([P, KC, Lc], fp8, name="kT_8", tag="kT")
            for mc in range(KC):
                ps = psum.tile([P, Lc], f32, tag="mm", name=f"ps_k_{mc}")
                for cx in range(KX):
                    nc.tensor.matmul(ps, wk_f[:, cx, mc * P:(mc + 1) * P],
                                     r(ctxT_f[:, cx, :]),
                                     start=(cx == 0), stop=(cx == KX - 1))
                nc.scalar.activation(kT_8[:, mc, :], ps, AF.Identity,
                                     bias=bk_t[:, mc:mc + 1])

            # vT [128(c), KC, Lc] = (ctx Wv + bv)^T
            vT_f = const.tile([P, KC, Lc], f32, name="vT_f", tag="vT")
            for mc in range(KC):
                ps = psum.tile([P, Lc], f32, tag="mm", name=f"ps_vt_{mc}")
                for cx in range(KX):
                    nc.tensor.matmul(ps, wv_f[:, cx, mc * P:(mc + 1) * P],
                                     r(ctxT_f[:, cx, :]),
                                     start=(cx == 0), stop=(cx == KX - 1))
                nc.scalar.activation(r(vT_f[:, mc, :]), ps, AF.Identity,
                                     bias=bv_t[:, mc:mc + 1])

            # G [128(c'), KC, Lc] = Wq kT, fp8 (DoubleRow pairs of c-tiles)
            G_8 = const.tile([P, KC, Lc], fp8, name="G_8", tag="G")
            for mg in range(KC):
                ps = psum.tile([P, Lc], f32, tag="mm", name=f"ps_g_{mg}")
                for u in range(KC // 2):
                    nc.tensor.matmul(ps,
                                     wqT_8[:, 2 * u:2 * u + 2,
                                           mg * P:(mg + 1) * P],
                                     kT_8[:, 2 * u:2 * u + 2, :],
                                     start=(u == 0), stop=(u == KC // 2 - 1),
                                     perf_mode=DR)
                nc.scalar.activation(G_8[:, mg, :], ps, AF.Copy)

            # bqk_s [128(lc), KL] = SCALE * kT^T bq   (per-lc exp bias)
            bqk_s = const.tile([P, KL], f32, name="bqk_s", tag="bqk")
            for ml in range(KL):
                ps = psum.tile([P, HT], f32, tag="mm", name=f"ps_bq_{ml}")
                for mc in range(KC):
                    nc.tensor.matmul(ps[:, 0:1],
                                     kT_8[:, mc, ml * P:(ml + 1) * P],
                                     bq_t[:, mc:mc + 1],
                                     start=(mc == 0), stop=(mc == KC - 1))
                nc.scalar.activation(bqk_s[:, ml:ml + 1], ps[:, 0:1],
                                     AF.Identity, scale=SCALE)

            # vW [128(lc), KL, C(c_out)] = (v + bv) Wo + 1 bo^T, fp8e4
            vW_b = const.tile([P, KL, C], fp8, name="vW_b", tag="vW")
            for ml in range(KL):
                ps = psum.tile([P, C], f32, tag="mm", name=f"ps_vw_{ml}")
                for mc in range(KC):
                    nc.tensor.matmul(ps, r(vT_f[:, mc, ml * P:(ml + 1) * P]),
                                     wo_f[:, mc, :],
                                     start=(mc == 0), stop=(mc == KC - 1))
                nc.vector.tensor_add(out=vW_b[:, ml, :], in0=ps, in1=bo_bc)

            # ---------------- phase B: stream over hw tiles ----------------
            # x-casts to fp8 run one tile ahead so they hide under the
            # previous tile's matmuls instead of gating simT
            x8_tiles = {}

            def emit_x8(h):
                x8 = work.tile([P, KC, HT], fp8, tag="x8", name=f"x8_{h}")
                nc.scalar.activation(x8[:], x_tiles[h][:].bitcast(f32), AF.Copy)
                x8_tiles[h] = x8

            def emit_yT(h, eT, rec_sb, x_f):
                # y [c_out, hw] = (vW^T eT) * recip + x  (fp8 DoubleRow)
                hs = slice(h * HT, (h + 1) * HT)
                y_sb = yout.tile([P, KC, HT], f32, tag="y", name=f"y_{h}")
                for mo in range(KC):
                    ps = psum.tile([P, HT], f32, tag="mmy", bufs=3,
                                   name=f"ps_y_{h}_{mo}")
                    for u in range(KL // 2):
                        nc.tensor.matmul(ps,
                                         vW_b[:, 2 * u:2 * u + 2,
                                              mo * P:(mo + 1) * P],
                                         eT[:, 2 * u:2 * u + 2, :],
                                         start=(u == 0), stop=(u == KL // 2 - 1),
                                         perf_mode=DR)
                    nc.vector.tensor_mul(out=y_sb[:, mo, :], in0=ps, in1=rec_sb)
                    nc.vector.tensor_add(out=y_sb[:, mo, :], in0=y_sb[:, mo, :],
                                         in1=x_f[:, mo, :].bitcast(f32))
                    nc.sync.dma_start(out=y_r[:, mo, hs], in_=y_sb[:, mo, :])

            emit_x8(0)
            prev = None

            for h in range(N_HT):
                hs = slice(h * HT, (h + 1) * HT)

                if h + 2 < N_HT and (h + 2) not in x_tiles:
                    x_p = xin.tile([P, KC, HT], f32r, tag="x_f",
                                   name=f"x_f_{h + 2}")
                    nc.sync.dma_start(
                        out=x_p, in_=x_r[:, :, (h + 2) * HT:(h + 3) * HT])
                    x_tiles[h + 2] = x_p
                if h + 1 < N_HT:
                    emit_x8(h + 1)

                x_f = x_tiles[h]
                x_8 = x8_tiles[h]

                # eT [lc, hw] = exp(scale * (G^T x) + scale * kT^T bq)
                eT = work.tile([P, KL, HT], fp8, tag="eT", name=f"eT_{h}")
                for ml in range(KL):
                    ps = psum.tile([P, HT], f32, tag="mm", name=f"ps_s_{h}_{ml}")
                    for u in range(KC // 2):
                        nc.tensor.matmul(ps,
                                         G_8[:, 2 * u:2 * u + 2,
                                             ml * P:(ml + 1) * P],
                                         x_8[:, 2 * u:2 * u + 2, :],
                                         start=(u == 0), stop=(u == KC // 2 - 1),
                                         perf_mode=DR)
                    nc.scalar.activation(eT[:, ml, :], ps, AF.Exp, scale=SCALE,
                                         bias=bqk_s[:, ml:ml + 1])

                # softmax denominator: ones^T @ eT -> broadcast -> 1/sum
                ps_sum = psum_s.tile([1, HT], f32, tag="sum", name=f"ps_sum_{h}")
                for ml in range(KL):
                    nc.tensor.matmul(ps_sum, ones_col, eT[:, ml, :],
                                     start=(ml == 0), stop=(ml == KL - 1))
                sum_sb = small.tile([1, HT], bf16, tag="sum_sb", name=f"sum_sb_{h}")
                nc.scalar.activation(sum_sb, ps_sum, AF.Copy)
                ps_bc = psum_bc.tile([P, HT], f32, tag="bc", name=f"ps_bc_{h}")
                nc.tensor.matmul(ps_bc, ones_row, sum_sb, start=True, stop=True)
                rec_sb = work.tile([P, HT], f32, tag="rec_sb", name=f"rec_sb_{h}")
                nc.vector.reciprocal_approx_fast(out=rec_sb, in_=ps_bc)

                # attn@V runs one tile behind, so the next tile's simT matmuls
                # fill the softmax-chain latency on the PE
                if prev is not None:
                    emit_yT(*prev)
                prev = (h, eT, rec_sb, x_f)
            emit_yT(*prev)

    nc.compile()
    return nc


def _get_compiled():
    if "nc" not in _cache:
        _cache["nc"] = _build_nc()
    return _cache["nc"]


def _make_in_maps(x, context, Wq, bq, Wk, bk, Wv, bv, Wo, bo):
    x = np.ascontiguousarray(np.asarray(x, dtype=np.float32))
    context = np.ascontiguousarray(np.asarray(context, dtype=np.float32))
    common = {
        "wq": np.ascontiguousarray(np.asarray(Wq, dtype=np.float32)),
        "wk": np.ascontiguousarray(np.asarray(Wk, dtype=np.float32)),
        "wv": np.ascontiguousarray(np.asarray(Wv, dtype=np.float32)),
        "wo": np.ascontiguousarray(np.asarray(Wo, dtype=np.float32)),
        "bq": np.ascontiguousarray(np.asarray(bq, dtype=np.float32)),
        "bk": np.ascontiguousarray(np.asarray(bk, dtype=np.float32)),
        "bv": np.ascontiguousarray(np.asarray(bv, dtype=np.float32)),
        "bo": np.ascontiguousarray(np.asarray(bo, dtype=np.float32)),
    }
    in_maps = []
    for b in range(B):
        m = dict(common)
        m["x"] = np.ascontiguousarray(x[b].reshape(C, HW))
        m["ctx"] = np.ascontiguousarray(context[b])
        in_maps.append(m)
    return in_maps


def _run(in_maps, trace=False):
    from concourse.bass_utils import run_bass_kernel_spmd
    nc = _get_compiled()
    return run_bass_kernel_spmd(nc, in_maps, core_ids=list(range(N_CORES)),
                                trace=trace)


def kernel(x, context, Wq, bq, Wk, bk, Wv, bv, Wo, bo):
    in_maps = _make_in_maps(x, context, Wq, bq, Wk, bk, Wv, bv, Wo, bo)
    res = _run(in_maps, trace=False)
    out = np.stack([res.results[b]["y"].reshape(C, HH, WW) for b in range(B)])
    return out.astype(np.float32)


# revision 56
# speedup vs baseline: 1.2116x; 1.2116x over previous
"""Cross-attention (single-head, residual) Bass/Tile kernel for Trainium2.

Problem: y = x + (softmax((x' Wq + bq)(ctx Wk + bk)^T / sqrt(C)) (ctx Wv + bv)) Wo + bo
  x: [B=8, C=512, H=64, W=64], context: [B=8, Lc=512, CTX=768]

Sharding: pure data-parallel over batch — one batch element per NeuronCore,
no collectives.

Algebraic restructuring (saves ~1/3 of the matmul work): with
  kT = (ctx Wk)^T            [C, Lc]
  G  = Wq kT                 [C, Lc]   (Wq folded into the key side)
  vW = (ctx Wv + bv) Wo      [Lc, C]   (Wo folded into the value side)
the streaming loop per hw-tile is only two matmul stages:
  simT = G^T-contracted-with-x:  simT[lc,hw] = sum_c' x[c',hw] G[c',lc]
  eT   = exp(scale*simT + scale*(kT^T bq))        (bq folded into ACT bias)
  y    = (vW^T eT) * (1/colsum(eT)) + bo + x      (softmax denom folded in)
bv is exact under the fold because softmax rows sum to 1.

Dtype strategy: HBM-resident tensors stay fp32 and feed the PE as float32r
(full rate at free-dim 512) — no cast passes exist.  Engine-produced matmul
operands are written f32r or bf16 during their mandatory PSUM evictions.
Accumulation is always fp32 in PSUM; residual add and output are fp32.
"""

import numpy as np

B = 8
C = 512
CTX = 768
Lc = 512
HH = 64
WW = 64
HW = HH * WW          # 4096
N_CORES = 8
P = 128
HT = 512              # hw tile (free-dim) width
N_HT = HW // HT       # 8
KC = C // P           # 4
KX = CTX // P         # 6
KL = Lc // P          # 4
SCALE = float(C) ** -0.5

_cache = {}


def _build_nc():
    import concourse.mybir as mybir
    import concourse.bass as bass
    import concourse.tile as tile
    from concourse import bacc
    from concourse.masks import make_identity

    f32 = mybir.dt.float32
    f32r = mybir.dt.float32r
    bf16 = mybir.dt.bfloat16
    fp8 = mybir.dt.float8e4
    AF = mybir.ActivationFunctionType
    DR = mybir.MatmulPerfMode.DoubleRow

    nc = bacc.Bacc("TRN2", target_bir_lowering=False, debug=False,
                   num_devices=N_CORES)

    x_d = nc.dram_tensor("x", [C, HW], f32r, kind="ExternalInput").ap()
    ctx_d = nc.dram_tensor("ctx", [Lc, CTX], f32, kind="ExternalInput").ap()
    wq_d = nc.dram_tensor("wq", [C, C], f32r, kind="ExternalInput").ap()
    wk_d = nc.dram_tensor("wk", [CTX, C], f32r, kind="ExternalInput").ap()
    wv_d = nc.dram_tensor("wv", [CTX, C], f32r, kind="ExternalInput").ap()
    wo_d = nc.dram_tensor("wo", [C, C], f32r, kind="ExternalInput").ap()
    bq_d = nc.dram_tensor("bq", [C], f32, kind="ExternalInput").ap()
    bk_d = nc.dram_tensor("bk", [C], f32, kind="ExternalInput").ap()
    bv_d = nc.dram_tensor("bv", [C], f32, kind="ExternalInput").ap()
    bo_d = nc.dram_tensor("bo", [C], f32, kind="ExternalInput").ap()
    y_d = nc.dram_tensor("y", [C, HW], f32, kind="ExternalOutput").ap()

    x_r = x_d.rearrange("(ko p) hw -> p ko hw", p=P)      # [128, 4, 4096]
    y_r = y_d.rearrange("(ko p) hw -> p ko hw", p=P)
    ctx_r = ctx_d.rearrange("(lo p) cx -> p lo cx", p=P)  # [128, 4, 768]
    wq_r = wq_d.rearrange("(ko p) c -> p ko c", p=P)      # [128, 4, 512]
    wk_r = wk_d.rearrange("(ko p) c -> p ko c", p=P)      # [128, 6, 512]
    wv_r = wv_d.rearrange("(ko p) c -> p ko c", p=P)
    wo_r = wo_d.rearrange("(ko p) c -> p ko c", p=P)

    def r(ap):  # feed fp32 SBUF data to the PE at full rate
        return ap.bitcast(mybir.dt.float32r)

    with tile.TileContext(nc) as tc:
        with (
            tc.tile_pool(name="const", bufs=1) as const,
            tc.tile_pool(name="xin", bufs=4) as xin,
            tc.tile_pool(name="work", bufs=2) as work,
            tc.tile_pool(name="yout", bufs=2) as yout,
            tc.tile_pool(name="small", bufs=3) as small,
            tc.tile_pool(name="psum", bufs=3, space="PSUM") as psum,
            tc.tile_pool(name="psum_s", bufs=1, space="PSUM") as psum_s,
            tc.tile_pool(name="psum_bc", bufs=1, space="PSUM") as psum_bc,
        ):
            # ---------------- DMAs (ordered: ctx feeds the PE first) --------
            ctx_f = const.tile([P, KL, CTX], f32, name="ctx_f", tag="ctx_f")
            for lo in range(KL):  # chunked so transposes start early
                nc.sync.dma_start(out=ctx_f[:, lo, :CTX // 2],
                                  in_=ctx_r[:, lo, :CTX // 2])
                nc.sync.dma_start(out=ctx_f[:, lo, CTX // 2:],
                                  in_=ctx_r[:, lo, CTX // 2:])
            wq_f = const.tile([P, KC, C], f32r, name="wq_f", tag="wq_f")
            wk_f = const.tile([P, KX, C], f32r, name="wk_f", tag="wk_f")
            wv_f = const.tile([P, KX, C], f32r, name="wv_f", tag="wv_f")
            wo_f = const.tile([P, KC, C], f32r, name="wo_f", tag="wo_f")
            nc.sync.dma_start(out=wq_f, in_=wq_r)
            nc.sync.dma_start(out=wk_f, in_=wk_r)
            nc.sync.dma_start(out=wv_f, in_=wv_r)
            nc.sync.dma_start(out=wo_f, in_=wo_r)

            x_tiles = {}
            for h in range(2):
                x_f = xin.tile([P, KC, HT], f32r, tag="x_f", name=f"x_f_{h}")
                nc.sync.dma_start(out=x_f, in_=x_r[:, :, h * HT:(h + 1) * HT])
                x_tiles[h] = x_f

            ident_f = const.tile([P, P], f32, name="ident_f", tag="ident")
            make_identity(nc, ident_f)
            ones_col = const.tile([P, 1], fp8, name="ones_col", tag="ones_c")
            nc.vector.memset(ones_col, 1.0)
            ones_row = const.tile([1, P], bf16, name="ones_row", tag="ones_r")
            nc.vector.memset(ones_row, 1.0)

            # biases (tiny scattered DMAs on the gpsimd queue; bq cast to fp8
            # by the gpsimd DGE so it pairs with the fp8 kT in the bqk matvec)
            bq_t = const.tile([P, KC], fp8, name="bq_t", tag="bq")
            bk_t = const.tile([P, KC], f32, name="bk_t", tag="bk")
            bv_t = const.tile([P, KC], f32, name="bv_t", tag="bv")
            with nc.allow_non_contiguous_dma(reason="tiny one-time bias loads"):
                nc.gpsimd.dma_start(out=bq_t, in_=bq_d.rearrange("(ko p) -> p ko", p=P))
                nc.gpsimd.dma_start(out=bk_t, in_=bk_d.rearrange("(ko p) -> p ko", p=P))
                nc.gpsimd.dma_start(out=bv_t, in_=bv_d.rearrange("(ko p) -> p ko", p=P))
            # bo broadcast across partitions (folded into vW: rows of attn sum
            # to 1, so attn @ (vW + 1 bo^T) = attn vW + bo exactly)
            bo_bc = const.tile([P, C], f32, name="bo_bc", tag="bo")
            bo_src = bass.AP(tensor=bo_d.tensor, offset=bo_d.offset,
                             ap=[[0, P]] + list(bo_d.ap))
            nc.gpsimd.dma_start(out=bo_bc, in_=bo_src)

            # ---------------- phase A ----------------
            # context transpose: ctxT [128(cx), KX, Lc]
            ctxT_f = const.tile([P, KX, Lc], f32, name="ctxT_f", tag="ctxT")
            for lo in range(KL):
                for cx in range(KX):
                    ps_t = psum.tile([P, P], f32, tag="mm", name=f"ps_t_{lo}_{cx}")
                    nc.tensor.transpose(ps_t, ctx_f[:, lo, cx * P:(cx + 1) * P],
                                        ident_f)
                    if cx % 2 == 0:
                        nc.scalar.activation(r(ctxT_f[:, cx, lo * P:(lo + 1) * P]),
                                             ps_t, AF.Copy)
                    else:
                        nc.vector.tensor_copy(
                            out=r(ctxT_f[:, cx, lo * P:(lo + 1) * P]), in_=ps_t)

            # Wq transpose: WqT [128(c), KC, C(c')], fp8
            wqT_8 = const.tile([P, KC, C], fp8, name="wqT_8", tag="wqT")
            for ko in range(KC):
                for mc in range(KC):
                    ps_t = psum.tile([P, P], f32, tag="mm", name=f"ps_w_{ko}_{mc}")
                    nc.tensor.transpose(
                        ps_t, wq_f[:, ko, mc * P:(mc + 1) * P].bitcast(f32), ident_f)
                    nc.vector.tensor_copy(
                        out=wqT_8[:, mc, ko * P:(ko + 1) * P], in_=ps_t)

            # kT [128(c), KC, Lc] = (ctx Wk + bk)^T, fp8 (K-side only shifts
            # logits; the V path stays full precision)
            kT_8 = const.tile([P, KC, Lc], fp8, name="kT_8", tag="kT")
            for mc in range(KC):
                ps = psum.tile([P, Lc], f32, tag="mm", name=f"ps_k_{mc}")
                for cx in range(KX):
                    nc.tensor.matmul(ps, wk_f[:, cx, mc * P:(mc + 1) * P],
                                     r(ctxT_f[:, cx, :]),
                                     start=(cx == 0), stop=(cx == KX - 1))
                nc.scalar.activation(kT_8[:, mc, :], ps, AF.Identity,
                                     bias=bk_t[:, mc:mc + 1])

            # vT [128(c), KC, Lc] = (ctx Wv + bv)^T
            vT_f = const.tile([P, KC, Lc], f32, name="vT_f", tag="vT")
            for mc in range(KC):
                ps = psum.tile([P, Lc], f32, tag="mm", name=f"ps_vt_{mc}")
                for cx in range(KX):
                    nc.tensor.matmul(ps, wv_f[:, cx, mc * P:(mc + 1) * P],
                                     r(ctxT_f[:, cx, :]),
                                     start=(cx == 0), stop=(cx == KX - 1))
                nc.scalar.activation(r(vT_f[:, mc, :]), ps, AF.Identity,
                                     bias=bv_t[:, mc:mc + 1])

            # G [128(c'), KC, Lc] = Wq kT, fp8 (DoubleRow pairs of c-tiles)
            G_8 = const.tile([P, KC, Lc], fp8, name="G_8", tag="G")
            for mg in range(KC):
                ps = psum.tile([P, Lc], f32, tag="mm", name=f"ps_g_{mg}")
                for u in range(KC // 2):
                    nc.tensor.matmul(ps,
                                     wqT_8[:, 2 * u:2 * u + 2,
                                           mg * P:(mg + 1) * P],
                                     kT_8[:, 2 * u:2 * u + 2, :],
                                     start=(u == 0), stop=(u == KC // 2 - 1),
                                     perf_mode=DR)
                nc.scalar.activation(G_8[:, mg, :], ps, AF.Copy)

            # bqk_s [128(lc), KL] = SCALE * kT^T bq   (per-lc exp bias)
            bqk_s = const.tile([P, KL], f32, name="bqk_s", tag="bqk")
            for ml in range(KL):
                ps = psum.tile([P, HT], f32, tag="mm", name=f"ps_bq_{ml}")
                for mc in range(KC):
                    nc.tensor.matmul(ps[:, 0:1],
                                     kT_8[:, mc, ml * P:(ml + 1) * P],
                                     bq_t[:, mc:mc + 1],
                                     start=(mc == 0), stop=(mc == KC - 1))
                nc.scalar.activation(bqk_s[:, ml:ml + 1], ps[:, 0:1],
                                     AF.Identity, scale=SCALE)

            # vW [128(lc), KL, C(c_out)] = (v + bv) Wo + 1 bo^T, fp8e4
            vW_b = const.tile([P, KL, C], fp8, name="vW_b", tag="vW")
            for ml in range(KL):
                ps = psum.tile([P, C], f32, tag="mm", name=f"ps_vw_{ml}")
                for mc in range(KC):
                    nc.tensor.matmul(ps, r(vT_f[:, mc, ml * P:(ml + 1) * P]),
                                     wo_f[:, mc, :],
                                     start=(mc == 0), stop=(mc == KC - 1))
                nc.vector.tensor_add(out=vW_b[:, ml, :], in0=ps, in1=bo_bc)

            # ---------------- phase B: stream over hw tiles ----------------
            # x-casts to fp8 run one tile ahead so they hide under the
            # previous tile's matmuls instead of gating simT
            x8_tiles = {}

            def emit_x8(h):
                # split the fp8 cast across ACT and DVE to balance engine load
                x8 = work.tile([P, KC, HT], fp8, tag="x8", name=f"x8_{h}")
                src = x_tiles[h][:].bitcast(f32)
                nc.scalar.activation(x8[:, :2, :], src[:, :2, :], AF.Copy)
                nc.vector.tensor_copy(out=x8[:, 2:, :], in_=src[:, 2:, :])
                x8_tiles[h] = x8

            def emit_yT(h, eT, rec_sb, x_f):
                # y [c_out, hw] = (vW^T eT) * recip + x  (fp8 DoubleRow)
                hs = slice(h * HT, (h + 1) * HT)
                y_sb = yout.tile([P, KC, HT], f32, tag="y", name=f"y_{h}")
                for mo in range(KC):
                    ps = psum.tile([P, HT], f32, tag="mmy", bufs=3,
                                   name=f"ps_y_{h}_{mo}")
                    for u in range(KL // 2):
                        nc.tensor.matmul(ps,
                                         vW_b[:, 2 * u:2 * u + 2,
                                              mo * P:(mo + 1) * P],
                                         eT[:, 2 * u:2 * u + 2, :],
                                         start=(u == 0), stop=(u == KL // 2 - 1),
                                         perf_mode=DR)
                    nc.vector.tensor_mul(out=y_sb[:, mo, :], in0=ps, in1=rec_sb)
                    nc.gpsimd.tensor_add(out=y_sb[:, mo, :], in0=y_sb[:, mo, :],
                                         in1=x_f[:, mo, :].bitcast(f32))
                    nc.sync.dma_start(out=y_r[:, mo, hs], in_=y_sb[:, mo, :])

            emit_x8(0)
            prev = None

            for h in range(N_HT):
                hs = slice(h * HT, (h + 1) * HT)

                if h + 2 < N_HT and (h + 2) not in x_tiles:
                    x_p = xin.tile([P, KC, HT], f32r, tag="x_f",
                                   name=f"x_f_{h + 2}")
                    nc.sync.dma_start(
                        out=x_p, in_=x_r[:, :, (h + 2) * HT:(h + 3) * HT])
                    x_tiles[h + 2] = x_p
                if h + 1 < N_HT:
                    emit_x8(h + 1)

                x_f = x_tiles[h]
                x_8 = x8_tiles[h]

                # eT [lc, hw] = exp(scale * (G^T x) + scale * kT^T bq)
                eT = work.tile([P, KL, HT], fp8, tag="eT", name=f"eT_{h}")
                for ml in range(KL):
                    ps = psum.tile([P, HT], f32, tag="mm", name=f"ps_s_{h}_{ml}")
                    for u in range(KC // 2):
                        nc.tensor.matmul(ps,
                                         G_8[:, 2 * u:2 * u + 2,
                                             ml * P:(ml + 1) * P],
                                         x_8[:, 2 * u:2 * u + 2, :],
                                         start=(u == 0), stop=(u == KC // 2 - 1),
                                         perf_mode=DR)
                    nc.scalar.activation(eT[:, ml, :], ps, AF.Exp, scale=SCALE,
                                         bias=bqk_s[:, ml:ml + 1])

                # softmax denominator: ones^T @ eT -> broadcast -> 1/sum
                ps_sum = psum_s.tile([1, HT], f32, tag="sum", name=f"ps_sum_{h}")
                for ml in range(KL):
                    nc.tensor.matmul(ps_sum, ones_col, eT[:, ml, :],
                                     start=(ml == 0), stop=(ml == KL - 1))
                sum_sb = small.tile([1, HT], bf16, tag="sum_sb", name=f"sum_sb_{h}")
                nc.scalar.activation(sum_sb, ps_sum, AF.Copy)
                ps_bc = psum_bc.tile([P, HT], f32, tag="bc", name=f"ps_bc_{h}")
                nc.tensor.matmul(ps_bc, ones_row, sum_sb, start=True, stop=True)
                rec_sb = work.tile([P, HT], f32, tag="rec_sb", name=f"rec_sb_{h}")
                nc.vector.reciprocal_approx_fast(out=rec_sb, in_=ps_bc)

                # attn@V runs one tile behind, so the next tile's simT matmuls
                # fill the softmax-chain latency on the PE
                if prev is not None:
                    emit_yT(*prev)
                prev = (h, eT, rec_sb, x_f)
            emit_yT(*prev)

    nc.compile()
    return nc


def _get_compiled():
    if "nc" not in _cache:
        _cache["nc"] = _build_nc()
    return _cache["nc"]


def _make_in_maps(x, context, Wq, bq, Wk, bk, Wv, bv, Wo, bo):
    x = np.ascontiguousarray(np.asarray(x, dtype=np.float32))
    context = np.ascontiguousarray(np.asarray(context, dtype=np.float32))
    common = {
        "wq": np.ascontiguousarray(np.asarray(Wq, dtype=np.float32)),
        "wk": np.ascontiguousarray(np.asarray(Wk, dtype=np.float32)),
        "wv": np.ascontiguousarray(np.asarray(Wv, dtype=np.float32)),
        "wo": np.ascontiguousarray(np.asarray(Wo, dtype=np.float32)),
        "bq": np.ascontiguousarray(np.asarray(bq, dtype=np.float32)),
        "bk": np.ascontiguousarray(np.asarray(bk, dtype=np.float32)),
        "bv": np.ascontiguousarray(np.asarray(bv, dtype=np.float32)),
        "bo": np.ascontiguousarray(np.asarray(bo, dtype=np.float32)),
    }
    in_maps = []
    for b in range(B):
        m = dict(common)
        m["x"] = np.ascontiguousarray(x[b].reshape(C, HW))
        m["ctx"] = np.ascontiguousarray(context[b])
        in_maps.append(m)
    return in_maps


def _run(in_maps, trace=False):
    from concourse.bass_utils import run_bass_kernel_spmd
    nc = _get_compiled()
    return run_bass_kernel_spmd(nc, in_maps, core_ids=list(range(N_CORES)),
                                trace=trace)


def kernel(x, context, Wq, bq, Wk, bk, Wv, bv, Wo, bo):
    in_maps = _make_in_maps(x, context, Wq, bq, Wk, bk, Wv, bv, Wo, bo)
    res = _run(in_maps, trace=False)
    out = np.stack([res.results[b]["y"].reshape(C, HH, WW) for b in range(B)])
    return out.astype(np.float32)
